# revision 24
# baseline (speedup 1.0000x reference)
"""LocallyConnected2d Trainium2 kernel — 2-location-packed variant.

Pack two adjacent output columns (w = 2k, 2k+1) into one matmul: their 3-tap
windows overlap into a 4-column window, so the contraction partitions become
p = m*32 + c with m = 0..3 the ABSOLUTE padded column offset (wcol = 2k + m)
and the stationary becomes [128, (l, o)] = 64 columns for the two locations.
l=0 uses taps m = 0..2 (rows 0..95), l=1 uses m = 1..3 (rows 32..127); the
complementary 32-row blocks are zero-padded in the DRAM weight image (the
matmul stationary AP must be a single free dim, so the padding rides the
load). Halves the matmul/LDWEIGHTS instruction count vs a 1-location kernel
(768 matmuls of [128,64]x[128,32] per core) and the 128-row fp8 LDWEIGHTS
gets fast-weight-load, pacing matmuls at ~29 ns. fp8e4m3 weights (rel err
1.1e-2 vs the 2e-2 gate), bf16 x, fp32 PSUM accumulation, bf16 stores.
Weight rows stream as halves on both HWDGE queues in consumption order.
"""

import sys

if "/opt/trn_rl_repo" not in sys.path:
    sys.path.insert(0, "/opt/trn_rl_repo")

import numpy as np
import ml_dtypes

BF16 = ml_dtypes.bfloat16
F8 = ml_dtypes.float8_e4m3

B = 32
C = 32
O = 32
H = W = 64
KK = 3
NCORES = 8
RP = H // NCORES      # output rows per core
RIN = RP + KK - 1     # input rows incl halo
NK = W // 2           # pair groups per row (32)
HF = NK * KK * O      # free elems per (h, l) block in w dram (= 3072)

_built = {}


def _build():
    if "nc" in _built:
        return _built["nc"]
    import concourse.tile as tile
    from concourse import bacc, mybir

    nc = bacc.Bacc("TRN2", target_bir_lowering=False, debug=False,
                   num_devices=NCORES)
    bf = mybir.dt.bfloat16
    f32 = mybir.dt.float32
    f8 = mybir.dt.float8e4
    xq = nc.dram_tensor("xq", [128, RIN, NK, B], bf, kind="ExternalInput")
    wp = nc.dram_tensor("wp", [128, RP, 2 * HF], f8, kind="ExternalInput")
    bp = nc.dram_tensor("bp", [128, RP, NK // 2], f32, kind="ExternalInput")
    op = nc.dram_tensor("op", [RP // 2, 128, NK * B], bf,
                        kind="ExternalOutput")

    with tile.TileContext(nc) as tc:
        with tc.tile_pool(name="xpool", bufs=1) as xpool, \
             tc.tile_pool(name="wpool", bufs=6) as wpool, \
             tc.tile_pool(name="opool", bufs=2) as opool, \
             tc.tile_pool(name="ppool", bufs=6, space="PSUM") as ppool:
            # weight tiles [128, (l, k*i*o)]; rows 96:128 of the l=0 half and
            # rows 0:32 of the l=1 half hold zeros (padded in DRAM).
            def load_w(h, eng=None):
                # halves ride both HWDGE queues concurrently; the kk<8
                # matmuls depend only on the first half (subtile deps)
                t = wpool.tile([128, 2 * HF], f8, tag="w")
                nc.sync.dma_start(t[:, 0:HF], wp.ap()[:, h, 0:HF])
                nc.scalar.dma_start(t[:, HF:2 * HF], wp.ap()[:, h, HF:2 * HF])
                return t

            # x chunks and w halves stream in consumption order across both
            # HWDGE queues.
            xqt = xpool.tile([128, RIN, NK, B], bf, tag="xq")
            nc.sync.dma_start(xqt[:, 0:3], xq.ap()[:, 0:3])
            wq = [load_w(0)]
            bt = xpool.tile([128, RP, NK // 2], f32, tag="bias")
            nc.scalar.dma_start(bt[:], bp.ap())
            wq.append(load_w(1))
            nc.sync.dma_start(xqt[:, 3:5], xq.ap()[:, 3:5])
            wq.append(load_w(2))
            wq.append(load_w(3))
            nc.sync.dma_start(xqt[:, 5:7], xq.ap()[:, 5:7])
            wq.append(load_w(4))
            wq.append(load_w(5))
            nc.sync.dma_start(xqt[:, 7:RIN], xq.ap()[:, 7:RIN])
            wq.append(load_w(6))
            wq.append(load_w(7))

            ot = None
            for h in range(RP):
                wth = wq[h]
                ps = ppool.tile([128, NK // 2, B], f32, tag="ps")
                for kk in range(NK // 2):
                    for i in range(KK):
                        for kp in range(2):
                            k = 2 * kk + kp
                            co = (k * KK + i) * 64
                            nc.tensor.matmul(
                                ps[64 * kp:64 * kp + 64, kk, :],
                                wth[:, co:co + 64],
                                xqt[:, h + i, k, :],
                                start=(i == 0),
                                stop=(i == KK - 1),
                                tile_position=(0, 64 * kp),
                                skip_group_check=True,
                            )
                if h % 2 == 0:
                    ot = opool.tile([128, 2, NK // 2, B], bf, tag="o")
                hw_half = NK // 2 * B
                nq = 2 if h == RP - 1 else 1
                for q in range(nq):  # final row streams out in halves
                    ks = (NK // 2) * q // nq
                    ke = (NK // 2) * (q + 1) // nq
                    nc.vector.tensor_add(
                        ot[:, h % 2, ks:ke], ps[:, ks:ke],
                        bt[:, h, ks:ke].unsqueeze(2).broadcast_to(
                            (128, ke - ks, B)))
                    nc.scalar.dma_start(
                        op.ap()[h // 2, :, (h % 2) * hw_half + ks * B:
                                           (h % 2) * hw_half + ke * B],
                        ot[:, h % 2, ks:ke])
    nc.compile()
    _built["nc"] = nc
    return nc


def prep_inputs(x, weights, bias):
    """Host-side shard + layout prep. Returns list of 8 in_maps."""
    x = np.asarray(x, dtype=np.float32)
    weights = np.asarray(weights, dtype=np.float32)
    bias = np.asarray(bias, dtype=np.float32)
    xpad = np.zeros((B, C, H + 2, W + 2), dtype=np.float32)
    xpad[:, :, 1:H + 1, 1:W + 1] = x
    xpad = xpad.astype(BF16)
    wf8 = weights.astype(F8)
    in_maps = []
    for d in range(NCORES):
        blk = xpad[:, :, RP * d:RP * d + RIN, :]          # [b, c, 10, 66]
        xprep = np.empty((128, RIN, NK, B), dtype=BF16)
        for m in range(4):
            xprep[32 * m:32 * m + 32] = blk[:, :, :, m:m + 64:2].transpose(
                1, 2, 3, 0)

        wd = wf8[RP * d:RP * d + RP]                      # [8, 64, 32, 32, 3, 3]
        wd = wd.reshape(RP, NK, 2, O, C, KK, KK)          # h, k, l, o, c, i, j
        # [128 rows = (m, c), h, (k, i, l, o)] with the dead taps zeroed
        wprep = np.zeros((128, RP, NK, KK, 2, O), dtype=F8)
        wA = wd[:, :, 0].transpose(5, 3, 0, 1, 4, 2)      # j, c, h, k, i, o
        wB = wd[:, :, 1].transpose(5, 3, 0, 1, 4, 2)
        wprep[0:96, :, :, :, 0, :] = wA.reshape(96, RP, NK, KK, O)
        wprep[32:128, :, :, :, 1, :] = wB.reshape(96, RP, NK, KK, O)
        wprep = wprep.reshape(128, RP, 2 * HF)

        # psum partition p = 64*kp + 32*l + o ; free kk ; w = 4*kk + 2*kp + l
        bd = bias[:, RP * d:RP * d + RP, :].reshape(O, RP, NK // 2, 2, 2)
        bprep = np.ascontiguousarray(bd.transpose(3, 4, 0, 1, 2)).reshape(
            128, RP, NK // 2)                              # (kp,l,o), h, kk
        in_maps.append({"xq": xprep, "wp": wprep, "bp": bprep})
    return in_maps


def assemble_output(results):
    """results: list of 8 dicts with 'op' [4, 128, 1024] -> full [B,O,H,W]."""
    out = np.empty((B, O, H, W), dtype=np.float32)
    for d in range(NCORES):
        arr = np.asarray(results[d]["op"]).astype(np.float32).reshape(
            RP // 2, 2, 2, O, 2, NK // 2, B)   # ck, kp, l, o, hh, kk, b
        out[:, :, RP * d:RP * d + RP, :] = (
            arr.transpose(6, 3, 0, 4, 5, 1, 2).reshape(B, O, RP, W))
    return out


def _ensure_ntff_hook():
    """The agent image's antenv lacks axon_hooks; inject it and register the
    ctypes NTFF hook (same recipe as trn_agent_boot.trn_boot)."""
    try:
        from antenv.axon_hooks import get_axon_ntff_profile_hook  # noqa: F401
        return
    except ImportError:
        pass
    import types
    import ctypes
    import contextlib

    mod = types.ModuleType("antenv.axon_hooks")
    mod._hook = None

    def set_axon_ntff_profile_hook(h):
        mod._hook = h

    def get_axon_ntff_profile_hook():
        return mod._hook

    mod.set_axon_ntff_profile_hook = set_axon_ntff_profile_hook
    mod.get_axon_ntff_profile_hook = get_axon_ntff_profile_hook
    sys.modules["antenv.axon_hooks"] = mod
    import antenv

    antenv.axon_hooks = mod

    so_path = "/opt/axon/libaxon_pjrt.so"
    try:
        lib = ctypes.CDLL(so_path)
    except OSError:
        return
    if not hasattr(lib, "axon_start_nrt_profile"):
        return
    lib.axon_start_nrt_profile.argtypes = [
        ctypes.POINTER(ctypes.c_int64), ctypes.c_size_t]
    lib.axon_start_nrt_profile.restype = ctypes.c_int64
    lib.axon_stop_nrt_profile.argtypes = [ctypes.c_char_p]
    lib.axon_stop_nrt_profile.restype = ctypes.c_int64

    @contextlib.contextmanager
    def _hook(output_dir, device_ids):
        import jax

        jax.devices()
        if device_ids:
            ids = (ctypes.c_int64 * len(device_ids))(*device_ids)
            rc = lib.axon_start_nrt_profile(ids, len(device_ids))
        else:
            rc = lib.axon_start_nrt_profile(None, 0)
        if rc != 0:
            raise RuntimeError(f"axon_start_nrt_profile rc={rc}")
        try:
            yield
        finally:
            n = lib.axon_stop_nrt_profile(str(output_dir).encode())
            print(f"ntff profile: {n} file(s) written to {output_dir}")

    mod.set_axon_ntff_profile_hook(_hook)



def run(inputs, trace=False, **kwargs):
    from concourse.bass_utils import run_bass_kernel_spmd

    if trace:
        _ensure_ntff_hook()
    nc = _build()
    in_maps = prep_inputs(inputs["x"], inputs["weights"], inputs["bias"])
    res = run_bass_kernel_spmd(nc, in_maps, list(range(NCORES)),
                               trace=trace, **kwargs)
    return assemble_output(res.results), res


def kernel(**inputs):
    out, _ = run(inputs)
    return out


# revision 25
# speedup vs baseline: 1.0138x; 1.0138x over previous
"""LocallyConnected2d Trainium2 kernel — 2-location-packed variant.

Pack two adjacent output columns (w = 2k, 2k+1) into one matmul: their 3-tap
windows overlap into a 4-column window, so the contraction partitions become
p = m*32 + c with m = 0..3 the ABSOLUTE padded column offset (wcol = 2k + m)
and the stationary becomes [128, (l, o)] = 64 columns for the two locations.
l=0 uses taps m = 0..2 (rows 0..95), l=1 uses m = 1..3 (rows 32..127); the
complementary 32-row blocks are zero-padded in the DRAM weight image (the
matmul stationary AP must be a single free dim, so the padding rides the
load). Halves the matmul/LDWEIGHTS instruction count vs a 1-location kernel
(768 matmuls of [128,64]x[128,32] per core) and the 128-row fp8 LDWEIGHTS
gets fast-weight-load, pacing matmuls at ~29 ns. fp8e4m3 weights (rel err
1.1e-2 vs the 2e-2 gate), bf16 x, fp32 PSUM accumulation, bf16 stores.
Weight rows stream as halves on both HWDGE queues in consumption order.
"""

import sys

if "/opt/trn_rl_repo" not in sys.path:
    sys.path.insert(0, "/opt/trn_rl_repo")

import numpy as np
import ml_dtypes

BF16 = ml_dtypes.bfloat16
F8 = ml_dtypes.float8_e4m3

B = 32
C = 32
O = 32
H = W = 64
KK = 3
NCORES = 8
RP = H // NCORES      # output rows per core
RIN = RP + KK - 1     # input rows incl halo
NK = W // 2           # pair groups per row (32)
HF = NK * KK * O      # free elems per (h, l) block in w dram (= 3072)

_built = {}


def _build():
    if "nc" in _built:
        return _built["nc"]
    import concourse.tile as tile
    from concourse import bacc, mybir

    nc = bacc.Bacc("TRN2", target_bir_lowering=False, debug=False,
                   num_devices=NCORES)
    bf = mybir.dt.bfloat16
    f32 = mybir.dt.float32
    f8 = mybir.dt.float8e4
    xq = nc.dram_tensor("xq", [128, RIN, NK, B], bf, kind="ExternalInput")
    wp = nc.dram_tensor("wp", [128, RP, 2 * HF], f8, kind="ExternalInput")
    bp = nc.dram_tensor("bp", [128, RP, NK // 2], f32, kind="ExternalInput")
    op = nc.dram_tensor("op", [RP // 2, 128, NK * B], bf,
                        kind="ExternalOutput")

    with tile.TileContext(nc) as tc:
        with tc.tile_pool(name="xpool", bufs=1) as xpool, \
             tc.tile_pool(name="wpool", bufs=8) as wpool, \
             tc.tile_pool(name="opool", bufs=2) as opool, \
             tc.tile_pool(name="ppool", bufs=6, space="PSUM") as ppool:
            # weight tiles [128, (l, k*i*o)]; rows 96:128 of the l=0 half and
            # rows 0:32 of the l=1 half hold zeros (padded in DRAM).
            def load_w(h, eng=None):
                # halves ride both HWDGE queues concurrently; the kk<8
                # matmuls depend only on the first half (subtile deps)
                t = wpool.tile([128, 2 * HF], f8, tag="w")
                nc.sync.dma_start(t[:, 0:HF], wp.ap()[:, h, 0:HF])
                nc.scalar.dma_start(t[:, HF:2 * HF], wp.ap()[:, h, HF:2 * HF])
                return t

            # x chunks and w halves stream in consumption order across both
            # HWDGE queues.
            xqt = xpool.tile([128, RIN, NK, B], bf, tag="xq")
            nc.sync.dma_start(xqt[:, 0:3], xq.ap()[:, 0:3])
            wq = [load_w(0)]
            bt = xpool.tile([128, RP, NK // 2], f32, tag="bias")
            nc.scalar.dma_start(bt[:], bp.ap())
            wq.append(load_w(1))
            nc.sync.dma_start(xqt[:, 3:5], xq.ap()[:, 3:5])
            wq.append(load_w(2))
            wq.append(load_w(3))
            nc.sync.dma_start(xqt[:, 5:7], xq.ap()[:, 5:7])
            wq.append(load_w(4))
            wq.append(load_w(5))
            nc.sync.dma_start(xqt[:, 7:RIN], xq.ap()[:, 7:RIN])
            wq.append(load_w(6))
            wq.append(load_w(7))

            ot = None
            for h in range(RP):
                wth = wq[h]
                ps = ppool.tile([128, NK // 2, B], f32, tag="ps")
                for kk in range(NK // 2):
                    for i in range(KK):
                        for kp in range(2):
                            k = 2 * kk + kp
                            co = (k * KK + i) * 64
                            nc.tensor.matmul(
                                ps[64 * kp:64 * kp + 64, kk, :],
                                wth[:, co:co + 64],
                                xqt[:, h + i, k, :],
                                start=(i == 0),
                                stop=(i == KK - 1),
                                tile_position=(0, 64 * kp),
                                skip_group_check=True,
                            )
                if h % 2 == 0:
                    ot = opool.tile([128, 2, NK // 2, B], bf, tag="o")
                hw_half = NK // 2 * B
                nq = 2 if h == RP - 1 else 1
                for q in range(nq):  # final row streams out in halves
                    ks = (NK // 2) * q // nq
                    ke = (NK // 2) * (q + 1) // nq
                    nc.vector.tensor_add(
                        ot[:, h % 2, ks:ke], ps[:, ks:ke],
                        bt[:, h, ks:ke].unsqueeze(2).broadcast_to(
                            (128, ke - ks, B)))
                    nc.scalar.dma_start(
                        op.ap()[h // 2, :, (h % 2) * hw_half + ks * B:
                                           (h % 2) * hw_half + ke * B],
                        ot[:, h % 2, ks:ke])
    nc.compile()
    _built["nc"] = nc
    return nc


def prep_inputs(x, weights, bias):
    """Host-side shard + layout prep. Returns list of 8 in_maps."""
    x = np.asarray(x, dtype=np.float32)
    weights = np.asarray(weights, dtype=np.float32)
    bias = np.asarray(bias, dtype=np.float32)
    xpad = np.zeros((B, C, H + 2, W + 2), dtype=np.float32)
    xpad[:, :, 1:H + 1, 1:W + 1] = x
    xpad = xpad.astype(BF16)
    wf8 = weights.astype(F8)
    in_maps = []
    for d in range(NCORES):
        blk = xpad[:, :, RP * d:RP * d + RIN, :]          # [b, c, 10, 66]
        xprep = np.empty((128, RIN, NK, B), dtype=BF16)
        for m in range(4):
            xprep[32 * m:32 * m + 32] = blk[:, :, :, m:m + 64:2].transpose(
                1, 2, 3, 0)

        wd = wf8[RP * d:RP * d + RP]                      # [8, 64, 32, 32, 3, 3]
        wd = wd.reshape(RP, NK, 2, O, C, KK, KK)          # h, k, l, o, c, i, j
        # [128 rows = (m, c), h, (k, i, l, o)] with the dead taps zeroed
        wprep = np.zeros((128, RP, NK, KK, 2, O), dtype=F8)
        wA = wd[:, :, 0].transpose(5, 3, 0, 1, 4, 2)      # j, c, h, k, i, o
        wB = wd[:, :, 1].transpose(5, 3, 0, 1, 4, 2)
        wprep[0:96, :, :, :, 0, :] = wA.reshape(96, RP, NK, KK, O)
        wprep[32:128, :, :, :, 1, :] = wB.reshape(96, RP, NK, KK, O)
        wprep = wprep.reshape(128, RP, 2 * HF)

        # psum partition p = 64*kp + 32*l + o ; free kk ; w = 4*kk + 2*kp + l
        bd = bias[:, RP * d:RP * d + RP, :].reshape(O, RP, NK // 2, 2, 2)
        bprep = np.ascontiguousarray(bd.transpose(3, 4, 0, 1, 2)).reshape(
            128, RP, NK // 2)                              # (kp,l,o), h, kk
        in_maps.append({"xq": xprep, "wp": wprep, "bp": bprep})
    return in_maps


def assemble_output(results):
    """results: list of 8 dicts with 'op' [4, 128, 1024] -> full [B,O,H,W]."""
    out = np.empty((B, O, H, W), dtype=np.float32)
    for d in range(NCORES):
        arr = np.asarray(results[d]["op"]).astype(np.float32).reshape(
            RP // 2, 2, 2, O, 2, NK // 2, B)   # ck, kp, l, o, hh, kk, b
        out[:, :, RP * d:RP * d + RP, :] = (
            arr.transpose(6, 3, 0, 4, 5, 1, 2).reshape(B, O, RP, W))
    return out


def _ensure_ntff_hook():
    """The agent image's antenv lacks axon_hooks; inject it and register the
    ctypes NTFF hook (same recipe as trn_agent_boot.trn_boot)."""
    try:
        from antenv.axon_hooks import get_axon_ntff_profile_hook  # noqa: F401
        return
    except ImportError:
        pass
    import types
    import ctypes
    import contextlib

    mod = types.ModuleType("antenv.axon_hooks")
    mod._hook = None

    def set_axon_ntff_profile_hook(h):
        mod._hook = h

    def get_axon_ntff_profile_hook():
        return mod._hook

    mod.set_axon_ntff_profile_hook = set_axon_ntff_profile_hook
    mod.get_axon_ntff_profile_hook = get_axon_ntff_profile_hook
    sys.modules["antenv.axon_hooks"] = mod
    import antenv

    antenv.axon_hooks = mod

    so_path = "/opt/axon/libaxon_pjrt.so"
    try:
        lib = ctypes.CDLL(so_path)
    except OSError:
        return
    if not hasattr(lib, "axon_start_nrt_profile"):
        return
    lib.axon_start_nrt_profile.argtypes = [
        ctypes.POINTER(ctypes.c_int64), ctypes.c_size_t]
    lib.axon_start_nrt_profile.restype = ctypes.c_int64
    lib.axon_stop_nrt_profile.argtypes = [ctypes.c_char_p]
    lib.axon_stop_nrt_profile.restype = ctypes.c_int64

    @contextlib.contextmanager
    def _hook(output_dir, device_ids):
        import jax

        jax.devices()
        if device_ids:
            ids = (ctypes.c_int64 * len(device_ids))(*device_ids)
            rc = lib.axon_start_nrt_profile(ids, len(device_ids))
        else:
            rc = lib.axon_start_nrt_profile(None, 0)
        if rc != 0:
            raise RuntimeError(f"axon_start_nrt_profile rc={rc}")
        try:
            yield
        finally:
            n = lib.axon_stop_nrt_profile(str(output_dir).encode())
            print(f"ntff profile: {n} file(s) written to {output_dir}")

    mod.set_axon_ntff_profile_hook(_hook)



def run(inputs, trace=False, **kwargs):
    from concourse.bass_utils import run_bass_kernel_spmd

    if trace:
        _ensure_ntff_hook()
    nc = _build()
    in_maps = prep_inputs(inputs["x"], inputs["weights"], inputs["bias"])
    res = run_bass_kernel_spmd(nc, in_maps, list(range(NCORES)),
                               trace=trace, **kwargs)
    return assemble_output(res.results), res


def kernel(**inputs):
    out, _ = run(inputs)
    return out
